# revision 4
# baseline (speedup 1.0000x reference)
"""Multi-head attention on 8 Trainium2 NeuronCores.

Problem: B=2, L=2048, D=1024, N=16 heads, H=64.
Sharding: core i -> batch (i // 4), heads [4*(i%4), 4*(i%4)+4).
Each core: QKV projections for its 4 heads, full-seq attention,
partial output projection. Host sums the 4 partial projections per batch.

v2: K=64 matmuls (logits, out-proj) run as CONCURRENT head-pairs via
64x128 PE row-tiling (tile_position auto-derived from base partitions):
the even head of a pair lives in SBUF partitions 0-63 (tile T0), the odd
head in partitions 64-127 (tile T8). Each pair-step produces one
[128,1024] psl tile (two 512-col halves in different PSUM banks, as
required for concurrent row tiles) and one EXP instruction covers both.

Device pipeline (per core), contraction dim always on partitions:
  QT[e,l] = sum_d wq[d,e] xT[d,l]     kt likewise (pair-packed)
  LT[t,f] pair = 2 concurrent K=64 row-tiled matmuls
  E = exp(LT pair)                    one ACT op per pair
  [O_un.T ; norm] = [V_n | 1].T @ E_n per head (M=65, full array)
  O.T = O_un.T * (1/norm)             odd head DMA-shifted to parts 64-127
  out[l,d] = (sum_{even h} O.T wo) + (sum_{odd h} O.T wo): two K=64
  row-tiled accumulators psa/psb, merged by a DVE add during evacuation.
"""

import numpy as np

B, L, D = 2, 2048, 1024
NHEADS, HDIM = 16, 64
NCORES = 8
HPC = 4  # heads per core
E = HPC * HDIM  # 256
DCH = D // 128  # 8 d-chunks
TCH = L // 128  # 16 t chunks
FB = 512  # f-block size (one pair-step covers FB cols for both heads)
NFB = L // FB  # 4
VW = HDIM + 1  # V' width per head (64 cols V + 1 ones col)

_CACHED_NC = None


def _build_nc():
    import concourse.mybir as mybir
    from concourse import bacc
    from concourse.tile import TileContext

    f32 = mybir.dt.float32
    f32r = mybir.dt.float32r
    bf16 = mybir.dt.bfloat16
    EXP = mybir.ActivationFunctionType.Exp

    nc = bacc.Bacc("TRN2", target_bir_lowering=False, num_devices=NCORES)

    xq = nc.declare_dram_parameter("xq", [D, L], bf16, isOutput=False)
    xk = nc.declare_dram_parameter("xk", [D, L], bf16, isOutput=False)
    xv = nc.declare_dram_parameter("xv", [D, L], bf16, isOutput=False)
    wq = nc.declare_dram_parameter("wq", [D, E], bf16, isOutput=False)
    wk = nc.declare_dram_parameter("wk", [D, E], bf16, isOutput=False)
    wv = nc.declare_dram_parameter("wv", [D, E], bf16, isOutput=False)
    wo = nc.declare_dram_parameter("wo", [E, D], bf16, isOutput=False)
    out = nc.declare_dram_parameter("out", [L, D], f32, isOutput=True)

    with TileContext(nc) as tc:
        with tc.tile_pool(name="persist", bufs=1) as cpool:
            # --- persistent SBUF tensors ---
            wq_sb = cpool.tile([128, DCH, E], bf16, tag="wq")
            wk_sb = cpool.tile([128, DCH, E], bf16, tag="wk")
            wv_sb = cpool.tile([128, DCH, E], bf16, tag="wv")
            # wo pair-packed: even head of pair ch in rows 0-63 of [:,ch,:],
            # odd head in rows 64-127
            wo_sb = cpool.tile([128, 2, D], bf16, tag="wo")
            qt_sb = cpool.tile([128, 2, L], f32r, tag="qt")
            kt_sb = cpool.tile([128, 2, L], f32r, tag="kt")
            v_sb = cpool.tile([128, TCH, HPC * VW], bf16, tag="v")
            # O.T pair-packed like wo_sb
            ont = cpool.tile([128, 2, L], bf16, tag="ont")

            # ---------------- Phase 1: Q projection ----------------
            with tc.tile_pool(name="xp", bufs=2) as xpool:

                def load_x(x_dram):
                    xt = xpool.tile([128, DCH, L], bf16, tag="x")
                    xr = x_dram.rearrange("(c p) l -> p c l", p=128)
                    for d in range(DCH):
                        nc.sync.dma_start(out=xt[:, d, :], in_=xr[:, d, :])
                    return xt

                with tc.tile_pool(name="psA", bufs=4, space="PSUM") as psA:
                    nc.sync.dma_start(
                        out=wq_sb[:],
                        in_=wq.rearrange("(c p) e -> p c e", p=128),
                    )
                    xtq = load_x(xq)
                    nc.vector.memset(
                        v_sb[:].rearrange("p t (n c) -> p t n c", n=HPC)[
                            :, :, :, HDIM : HDIM + 1
                        ],
                        1.0,
                    )
                    nc.sync.dma_start(
                        out=wk_sb[:],
                        in_=wk.rearrange("(c p) e -> p c e", p=128),
                    )
                    xtk = load_x(xk)

                    # QT: d-outer, (e,lb) grid in two passes of 4 banks
                    grid = [(e, lb) for e in range(2) for lb in range(L // 512)]
                    for half in range(2):
                        cells = grid[half * 4 : half * 4 + 4]
                        pss = [
                            psA.tile(
                                [128, 512], f32, tag="ps", name=f"q_{half}_{i}"
                            )
                            for i in range(len(cells))
                        ]
                        for d in range(DCH):
                            for ps, (e, lb) in zip(pss, cells):
                                nc.tensor.matmul(
                                    ps[:],
                                    wq_sb[:, d, e * 128 : (e + 1) * 128],
                                    xtq[:, d, lb * 512 : (lb + 1) * 512],
                                    start=(d == 0),
                                    stop=(d == DCH - 1),
                                )
                        for ps, (e, lb) in zip(pss, cells):
                            nc.vector.tensor_copy(
                                out=qt_sb[:, e, lb * 512 : (lb + 1) * 512],
                                in_=ps[:],
                            )

                    xtv = load_x(xv)  # reuses xq's slot
                    nc.sync.dma_start(
                        out=wv_sb[:],
                        in_=wv.rearrange("(c p) e -> p c e", p=128),
                    )
                    # wo pair-packed load: row = (2*ch + par)*64 + h
                    wor = wo.rearrange(
                        "(ch par h) d -> h ch par d", ch=2, par=2
                    )
                    nc.sync.dma_start(
                        out=wo_sb[0:64, :, :], in_=wor[:, :, 0, :]
                    )
                    nc.sync.dma_start(
                        out=wo_sb[64:128, :, :], in_=wor[:, :, 1, :]
                    )

                # ---- Phase 2: ACT-paced pump over (fb, pair, t) steps ----
                PREF = 10
                from collections import deque

                with (
                    tc.tile_pool(name="psL", bufs=2, space="PSUM") as psL,
                    tc.tile_pool(name="ep", bufs=PREF) as epool,
                    tc.tile_pool(name="rp", bufs=2) as rpool,
                    tc.tile_pool(name="bp", bufs=2) as bpool,
                    tc.tile_pool(name="op", bufs=3) as opool,
                ):
                    blocks = [(fb, ch) for fb in range(NFB) for ch in range(2)]
                    cursor = [0]
                    pslq = deque()
                    etq = deque()

                    def pump_logits():
                        k = cursor[0]
                        if k >= len(blocks) * TCH:
                            return
                        cursor[0] += 1
                        fb, ch = blocks[k // TCH]
                        t = k % TCH
                        f0 = fb * FB
                        psl = psL.tile(
                            [128, 2 * FB], f32, tag="psl", name=f"psl_{k}"
                        )
                        # concurrent row-tiled pair: even head (T0) /
                        # odd head (T8); outputs in different PSUM banks
                        nc.tensor.matmul(
                            psl[:, 0:FB],
                            kt_sb[0:64, ch, t * 128 : (t + 1) * 128],
                            qt_sb[0:64, ch, f0 : f0 + FB],
                            start=True,
                            stop=True,
                        )
                        nc.tensor.matmul(
                            psl[:, FB : 2 * FB],
                            kt_sb[64:128, ch, t * 128 : (t + 1) * 128],
                            qt_sb[64:128, ch, f0 : f0 + FB],
                            start=True,
                            stop=True,
                        )
                        pslq.append((k, psl))

                    def pump_exp():
                        if not pslq:
                            return
                        k, psl = pslq.popleft()
                        et = epool.tile(
                            [128, 2 * FB], bf16, tag="e", name=f"et_{k}"
                        )
                        nc.scalar.activation(et[:], psl[:], EXP)
                        etq.append(et)

                    def pump():
                        pump_logits()
                        pump_exp()

                    pending = []
                    psW_holder = [None, None]

                    def outproj_group(lc, db):
                        pool_a, pool_b = psW_holder[0], psW_holder[1]
                        psa = pool_a.tile(
                            [128, 512], f32, tag="wa", name=f"pa_{lc}_{db}"
                        )
                        psb = pool_b.tile(
                            [128, 512], f32, tag="wb", name=f"pb_{lc}_{db}"
                        )
                        lsl = slice(lc * 128, (lc + 1) * 128)
                        dsl = slice(db * 512, (db + 1) * 512)
                        # interleave T0/T8 so pairs run concurrently
                        nc.tensor.matmul(
                            psa[:], ont[0:64, 0, lsl], wo_sb[0:64, 0, dsl],
                            start=True, stop=False,
                        )
                        nc.tensor.matmul(
                            psb[:], ont[64:128, 0, lsl], wo_sb[64:128, 0, dsl],
                            start=True, stop=False,
                        )
                        nc.tensor.matmul(
                            psa[:], ont[0:64, 1, lsl], wo_sb[0:64, 1, dsl],
                            start=False, stop=True,
                        )
                        nc.tensor.matmul(
                            psb[:], ont[64:128, 1, lsl], wo_sb[64:128, 1, dsl],
                            start=False, stop=True,
                        )
                        ota = opool.tile(
                            [128, 512], f32, tag="oa", name=f"ota_{lc}_{db}"
                        )
                        nc.vector.tensor_copy(out=ota[:], in_=psa[:])
                        ot = opool.tile(
                            [128, 512], f32, tag="o", name=f"ot_{lc}_{db}"
                        )
                        nc.vector.tensor_add(ot[:], ota[:], psb[:])
                        nc.sync.dma_start(
                            out=out[lsl, dsl],
                            in_=ot[:],
                        )

                    # KT: e-chunk (=pair) 0 plain; e=1 interleaved with the
                    # first PREF logits+exp so the ACT spins up while KT runs
                    def kt_e(e, psK, interleave):
                        cells = list(range(L // 512))
                        pss = [
                            psK.tile(
                                [128, 512], f32, tag="psk", name=f"k_{e}_{i}"
                            )
                            for i in cells
                        ]
                        for d in range(DCH):
                            for ps, lb in zip(pss, cells):
                                nc.tensor.matmul(
                                    ps[:],
                                    wk_sb[:, d, e * 128 : (e + 1) * 128],
                                    xtk[:, d, lb * 512 : (lb + 1) * 512],
                                    start=(d == 0),
                                    stop=(d == DCH - 1),
                                )
                            if interleave and d % 2 == 1:
                                pump()
                        for ps, lb in zip(pss, cells):
                            nc.vector.tensor_copy(
                                out=kt_sb[:, e, lb * 512 : (lb + 1) * 512],
                                in_=ps[:],
                            )

                    with tc.tile_pool(name="psK", bufs=4, space="PSUM") as psK:
                        kt_e(0, psK, False)
                        kt_e(1, psK, True)
                        while cursor[0] < PREF:
                            pump()

                    # V projection in 2-bank passes, interleaved into
                    # block 0's t-loop
                    psO_cm = tc.tile_pool(name="psO", bufs=2, space="PSUM")
                    psO = psO_cm.__enter__()
                    psV_cm = tc.tile_pool(name="psV", bufs=2, space="PSUM")
                    psV = psV_cm.__enter__()

                    def v_pass(p):
                        ts2 = range(p * 2, p * 2 + 2)
                        vps = [
                            psV.tile(
                                [128, E], f32, tag="psv", name=f"v_{p}_{i}"
                            )
                            for i in range(2)
                        ]
                        for d in range(DCH):
                            for ps, t in zip(vps, ts2):
                                nc.tensor.matmul(
                                    ps[:],
                                    xtv[:, d, t * 128 : (t + 1) * 128],
                                    wv_sb[:, d, :],
                                    start=(d == 0),
                                    stop=(d == DCH - 1),
                                )
                        for ps, t in zip(vps, ts2):
                            nc.vector.tensor_copy(
                                out=v_sb[:, t, :].rearrange(
                                    "p (n c) -> p n c", n=HPC
                                )[:, :, 0:HDIM],
                                in_=ps[:].rearrange("p (n c) -> p n c", n=HPC),
                            )

                    for bi, (fb, ch) in enumerate(blocks):
                        f0 = fb * FB
                        h0, h1 = 2 * ch, 2 * ch + 1
                        pso_a = psO.tile(
                            [VW, FB], f32, tag="pso", name=f"psoa_{bi}"
                        )
                        pso_b = psO.tile(
                            [VW, FB], f32, tag="pso", name=f"psob_{bi}"
                        )
                        for t in range(TCH):
                            if bi == 0 and t % 2 == 0:
                                v_pass(t // 2)
                            pump()
                            et = etq.popleft()
                            nc.tensor.matmul(
                                pso_a[:],
                                v_sb[:, t, h0 * VW : (h0 + 1) * VW],
                                et[:, 0:FB],
                                start=(t == 0),
                                stop=(t == TCH - 1),
                            )
                            nc.tensor.matmul(
                                pso_b[:],
                                v_sb[:, t, h1 * VW : (h1 + 1) * VW],
                                et[:, FB : 2 * FB],
                                start=(t == 0),
                                stop=(t == TCH - 1),
                            )
                            if t % 2 == 1 and pending:
                                outproj_group(*pending.pop(0))

                        if bi == 0:
                            psV_cm.__exit__(None, None, None)
                            psW_cm = tc.tile_pool(
                                name="psW", bufs=1, space="PSUM"
                            )
                            psW_holder[0] = psW_cm.__enter__()
                            psW2_cm = tc.tile_pool(
                                name="psW2", bufs=1, space="PSUM"
                            )
                            psW_holder[1] = psW2_cm.__enter__()

                        # --- normalize both heads of the pair ---
                        stage_a = rpool.tile(
                            [VW, FB], f32, tag="ra", name=f"sta_{bi}"
                        )
                        nc.vector.tensor_copy(out=stage_a[:], in_=pso_a[:])
                        stage_b = rpool.tile(
                            [VW, FB], f32, tag="rb", name=f"stb_{bi}"
                        )
                        nc.vector.tensor_copy(out=stage_b[:], in_=pso_b[:])
                        # norm rows -> [32, x] blocks, recip, back to rows
                        FB32 = FB // 32
                        rsq = rpool.tile([32, 4 * FB32], f32, tag="rsq")
                        nc.sync.dma_start(
                            out=rsq[:, 0:FB32], in_=stage_a[HDIM : HDIM + 1, :]
                        )
                        nc.sync.dma_start(
                            out=rsq[:, FB32 : 2 * FB32],
                            in_=stage_b[HDIM : HDIM + 1, :],
                        )
                        with nc.allow_low_precision(reason="softmax recip"):
                            nc.vector.reciprocal(
                                out=rsq[:, 2 * FB32 : 4 * FB32],
                                in_=rsq[:, 0 : 2 * FB32],
                            )
                        rt2 = rpool.tile([1, 2 * FB], f32, tag="r2")
                        nc.sync.dma_start(
                            out=rt2[:, 0:FB], in_=rsq[:, 2 * FB32 : 3 * FB32]
                        )
                        nc.sync.dma_start(
                            out=rt2[:, FB : 2 * FB],
                            in_=rsq[:, 3 * FB32 : 4 * FB32],
                        )
                        bt = bpool.tile([64, 2 * FB], f32, tag="b")
                        nc.gpsimd.partition_broadcast(
                            bt[:, 0:FB], rt2[:, 0:FB], channels=64
                        )
                        nc.gpsimd.partition_broadcast(
                            bt[:, FB : 2 * FB], rt2[:, FB : 2 * FB], channels=64
                        )
                        nc.vector.tensor_mul(
                            out=ont[0:HDIM, ch, f0 : f0 + FB],
                            in0=stage_a[0:HDIM, :],
                            in1=bt[:, 0:FB],
                        )
                        # odd head: multiply at partitions 0-63, then
                        # DMA-shift the bf16 result to partitions 64-127
                        ot2 = rpool.tile([64, FB], bf16, tag="o2", name=f"o2_{bi}")
                        nc.vector.tensor_mul(
                            out=ot2[:],
                            in0=stage_b[0:HDIM, :],
                            in1=bt[:, FB : 2 * FB],
                        )
                        nc.sync.dma_start(
                            out=ont[64:128, ch, f0 : f0 + FB], in_=ot2[:]
                        )

                        if ch == 1:
                            pending += [
                                (lc, db)
                                for lc in range(
                                    fb * (FB // 128), (fb + 1) * (FB // 128)
                                )
                                for db in range(D // 512)
                            ]

                    # drain the last f-block's out-projection
                    for lc, db in pending:
                        outproj_group(lc, db)
                    psW2_cm.__exit__(None, None, None)
                    psW_cm.__exit__(None, None, None)
                    psO_cm.__exit__(None, None, None)

    nc.compile()
    return nc


def _get_nc():
    global _CACHED_NC
    if _CACHED_NC is None:
        _CACHED_NC = _build_nc()
    return _CACHED_NC


def _make_in_maps(query_input, key_input, value_input, Wq, Wk, Wv, Wo):
    import ml_dtypes

    bf16 = ml_dtypes.bfloat16
    scale = np.float32(HDIM) ** np.float32(-0.5)

    xT = {}
    for b in range(B):
        xT[("q", b)] = np.ascontiguousarray(query_input[b].T).astype(bf16)
        xT[("k", b)] = np.ascontiguousarray(key_input[b].T).astype(bf16)
        xT[("v", b)] = np.ascontiguousarray(value_input[b].T).astype(bf16)

    in_maps = []
    for core in range(NCORES):
        b = core // 4
        g = core % 4
        hs = slice(g * HPC, (g + 1) * HPC)
        in_maps.append(
            {
                "xq": xT[("q", b)],
                "xk": xT[("k", b)],
                "xv": xT[("v", b)],
                "wq": np.ascontiguousarray(
                    (Wq[:, hs, :] * scale).reshape(D, E)
                ).astype(bf16),
                "wk": np.ascontiguousarray(Wk[:, hs, :].reshape(D, E)).astype(bf16),
                "wv": np.ascontiguousarray(Wv[:, hs, :].reshape(D, E)).astype(bf16),
                "wo": np.ascontiguousarray(Wo[hs].reshape(E, D)).astype(bf16),
            }
        )
    return in_maps


def _combine(results):
    out = np.empty((B, L, D), dtype=np.float32)
    for b in range(B):
        acc = results[b * 4]["out"].astype(np.float32)
        for g in range(1, 4):
            acc = acc + results[b * 4 + g]["out"]
        out[b] = acc
    return out


def kernel(query_input, key_input, value_input, Wq, Wk, Wv, Wo):
    from concourse.bass_utils import run_bass_kernel_spmd

    nc = _get_nc()
    in_maps = _make_in_maps(query_input, key_input, value_input, Wq, Wk, Wv, Wo)
    res = run_bass_kernel_spmd(nc, in_maps, core_ids=list(range(NCORES)))
    return _combine(res.results)


if __name__ == "__main__":
    rng = np.random.default_rng(0)
    inputs = {
        "query_input": rng.standard_normal((B, L, D), dtype=np.float32),
        "key_input": rng.standard_normal((B, L, D), dtype=np.float32),
        "value_input": rng.standard_normal((B, L, D), dtype=np.float32),
        "Wq": rng.standard_normal((D, NHEADS, HDIM), dtype=np.float32) * 0.03,
        "Wk": rng.standard_normal((D, NHEADS, HDIM), dtype=np.float32) * 0.03,
        "Wv": rng.standard_normal((D, NHEADS, HDIM), dtype=np.float32) * 0.03,
        "Wo": rng.standard_normal((NHEADS, HDIM, D), dtype=np.float32) * 0.03,
    }
    out = kernel(**inputs)
    print("kernel output", out.shape, out.dtype, float(np.abs(out).mean()))


# revision 14
# speedup vs baseline: 1.2263x; 1.2263x over previous
"""Multi-head attention on 8 Trainium2 NeuronCores.

Problem: B=2, L=2048, D=1024, N=16 heads, H=64.
Sharding: core i -> batch (i // 4), heads [4*(i%4), 4*(i%4)+4).
Each core: QKV projections for its 4 heads, full-seq attention,
partial output projection. Host sums the 4 partial projections per batch.

Device pipeline (per core), all layouts chosen so the contraction dim is
on partitions (no on-device transposes; host passes x pre-transposed):
  QT[e,l] = sum_d wq[d,e] xT[d,l]      (lhsT=wq chunk, rhs=xT chunk)
  KT[e,l] likewise; V[t,e] = sum_d xT[d,t-chunk] wv[d,e] (lhsT=xT, rhs=wv)
  LT[t,f] = sum_h KT[h,t] QT[h,f]      (per head)
  E = exp(LT)                          (ACT, PSUM->SBUF)
  [O_un.T ; norm] = [V_n | 1].T @ E:   lhsT=[V_n|ones][t,65], rhs=E[t,f]
  O.T = O_un.T * (1/norm) broadcast    (rank-1 matmul + 64-lane recip)
  out[l,d] += sum_h O.T[h,l-chunk] wo[h,d]

K=64 matmuls run at half rate on TRN2, so all contraction-64 matmuls
(logits, out-proj) are padded to K=128: the stationary operand keeps its
head's 64 rows and zeros elsewhere, making the moving operand's other
rows irrelevant. Matmuls run as float32r (full PE rate, ~TF32 rounding);
x and all weights are bf16 host-cast.
"""

import numpy as np

B, L, D = 2, 2048, 1024
NHEADS, HDIM = 16, 64
NCORES = 8
HPC = 4  # heads per core
E = HPC * HDIM  # 256
DCH = D // 128  # 8 d-chunks
TCH = L // 128  # 16 t/l chunks
FB = 1024  # f-block size in attention phase
NFB = L // FB
VW = HDIM + 1  # V' width per head (64 cols V + 1 ones col)

_CACHED_NC = None


def _build_nc():
    import concourse.mybir as mybir
    from concourse import bacc
    from concourse.tile import TileContext

    f32 = mybir.dt.float32
    f32r = mybir.dt.float32r
    bf16 = mybir.dt.bfloat16
    EXP = mybir.ActivationFunctionType.Exp

    nc = bacc.Bacc("TRN2", target_bir_lowering=False, num_devices=NCORES)

    xq = nc.declare_dram_parameter("xq", [D, L], bf16, isOutput=False)
    xk = nc.declare_dram_parameter("xk", [D, L], bf16, isOutput=False)
    xv = nc.declare_dram_parameter("xv", [D, L], bf16, isOutput=False)
    wq = nc.declare_dram_parameter("wq", [D, E], bf16, isOutput=False)
    wk = nc.declare_dram_parameter("wk", [D, E], bf16, isOutput=False)
    wv = nc.declare_dram_parameter("wv", [D, E], bf16, isOutput=False)
    wo = nc.declare_dram_parameter("wo", [E, D], bf16, isOutput=False)
    # out is stored TRANSPOSED [D, L]; host transposes back. This lets the
    # out-projection run as out.T[d,l] = sum_e wo[e,d] ont[e,l] with TWO
    # heads stacked in the K dim (no zero padding): 2 matmuls per tile
    # instead of 4.
    out = nc.declare_dram_parameter("out", [D, L], f32, isOutput=True)

    with TileContext(nc) as tc:
        with tc.tile_pool(name="persist", bufs=1) as cpool:
            # --- persistent SBUF tensors ---
            wq_sb = cpool.tile([128, DCH, E], bf16, tag="wq")
            wk_sb = cpool.tile([128, DCH, E], bf16, tag="wk")
            wv_sb = cpool.tile([128, DCH, E], bf16, tag="wv")
            # wo pair-stacked: pair ch holds head 2ch in rows 0-63 and head
            # 2ch+1 in rows 64-127 -> K=128 out-proj with no padding
            wo_sb = cpool.tile([128, 2, D], bf16, tag="wo")
            qt_sb = cpool.tile([128, 2, L], f32r, tag="qt")
            # kt zero-padded per head: head n's data in rows (n%2)*64..+64,
            # zeros in the other 64 rows -> K=128 logits at full rate
            kt_z = cpool.tile([128, HPC, L], f32r, tag="ktz")
            v_sb = cpool.tile([128, TCH, HPC * VW], bf16, tag="v")
            # O.T pair-stacked like wo_sb
            ont = cpool.tile([128, 2, L], bf16, tag="ont")

            # ---------------- Phase 1: Q and V projections ----------------
            # x pool spans both phases (xk is consumed by the KT projection
            # that runs inside the attention scope).
            with tc.tile_pool(name="xp", bufs=2) as xpool:

                def load_x(x_dram):
                    xt = xpool.tile([128, DCH, L], bf16, tag="x")
                    xr = x_dram.rearrange("(c p) l -> p c l", p=128)
                    for d in range(DCH):
                        nc.sync.dma_start(out=xt[:, d, :], in_=xr[:, d, :])
                    return xt

                with tc.tile_pool(name="psA", bufs=4, space="PSUM") as psA:
                    nc.sync.dma_start(
                        out=wq_sb[:],
                        in_=wq.rearrange("(c p) e -> p c e", p=128),
                    )
                    xtq = load_x(xq)
                    # tiny constants early: zero column for kt_z stripes
                    # (DVE work during the Q projection)
                    cst = np.concatenate(
                        [
                            np.ones((128, 64), np.float32),
                            np.zeros((128, 1), np.float32),
                        ],
                        axis=1,
                    )
                    cst_dram = nc.inline_tensor(cst, name="cst")
                    cst_src = cst_dram.ap().bitcast(f32r)
                    zero_sb = cpool.tile([128, 1], f32r, tag="zero")
                    nc.sync.dma_start(out=zero_sb[:], in_=cst_src[:, 64:65])
                    for zn in range(HPC):
                        lo = 64 if zn % 2 == 0 else 0
                        nc.vector.tensor_copy(
                            out=kt_z[lo : lo + 64, zn, :],
                            in_=zero_sb[lo : lo + 64, 0:1].to_broadcast(
                                (64, L)
                            ),
                        )


                    nc.sync.dma_start(
                        out=wk_sb[:],
                        in_=wk.rearrange("(c p) e -> p c e", p=128),
                    )
                    xtk = load_x(xk)

                    # QT: d-outer, (e,lb) grid in two passes of 4 banks
                    grid = [(e, lb) for e in range(2) for lb in range(L // 512)]
                    for half in range(2):
                        cells = grid[half * 4 : half * 4 + 4]
                        pss = [
                            psA.tile(
                                [128, 512], f32, tag="ps", name=f"q_{half}_{i}"
                            )
                            for i in range(len(cells))
                        ]
                        for d in range(DCH):
                            for ps, (e, lb) in zip(pss, cells):
                                nc.tensor.matmul(
                                    ps[:],
                                    wq_sb[:, d, e * 128 : (e + 1) * 128],
                                    xtq[:, d, lb * 512 : (lb + 1) * 512],
                                    start=(d == 0),
                                    stop=(d == DCH - 1),
                                )
                        for ps, (e, lb) in zip(pss, cells):
                            nc.vector.tensor_copy(
                                out=qt_sb[:, e, lb * 512 : (lb + 1) * 512],
                                in_=ps[:],
                            )

                    xtv = load_x(xv)  # reuses xq's slot
                    nc.vector.memset(
                        v_sb[:].rearrange("p t (n c) -> p t n c", n=HPC)[
                            :, :, :, HDIM : HDIM + 1
                        ],
                        1.0,
                    )
                    nc.sync.dma_start(
                        out=wv_sb[:],
                        in_=wv.rearrange("(c p) e -> p c e", p=128),
                    )
                    # pair-stacked wo: row index = (2*ch + par)*64 + h
                    wor = wo.rearrange(
                        "(ch par h) d -> h ch par d", ch=2, par=2
                    )
                    nc.sync.dma_start(
                        out=wo_sb[0:64, :, :], in_=wor[:, :, 0, :]
                    )
                    nc.sync.dma_start(
                        out=wo_sb[64:128, :, :], in_=wor[:, :, 1, :]
                    )

                # ---- Phase 2(+KT,+out-proj): ACT-paced global pump ----
                # The logits->exp stream runs through a global cursor with
                # ~PREF tiles of lookahead, so the ACT engine never stalls
                # on PE hiccups, head boundaries, or the KT projection
                # (which runs here, interleaved, on its own PSUM pool).
                PREF = 10
                from collections import deque

                with (
                    tc.tile_pool(name="psL", bufs=2, space="PSUM") as psL,
                    tc.tile_pool(name="ep", bufs=PREF) as epool,
                    tc.tile_pool(name="rp", bufs=2) as rpool,
                    tc.tile_pool(name="bp", bufs=2) as bpool,
                    tc.tile_pool(name="op", bufs=3) as opool,
                ):
                    heads = [(fb, n) for fb in range(NFB) for n in range(HPC)]
                    cursor = [0]
                    pslq = deque()
                    etq = deque()

                    def pump_logits():
                        k = cursor[0]
                        if k >= len(heads) * TCH:
                            return
                        cursor[0] += 1
                        fb, n = heads[k // TCH]
                        t = k % TCH
                        psl = psL.tile(
                            [128, FB], f32, tag="psl", name=f"psl_{k}"
                        )
                        ch, f0 = n // 2, fb * FB
                        for h2 in range(FB // 512):
                            nc.tensor.matmul(
                                psl[:, h2 * 512 : (h2 + 1) * 512],
                                kt_z[:, n, t * 128 : (t + 1) * 128],
                                qt_sb[
                                    :, ch, f0 + h2 * 512 : f0 + (h2 + 1) * 512
                                ],
                                start=True,
                                stop=True,
                            )
                        pslq.append((k, psl))

                    def pump_exp():
                        if not pslq:
                            return
                        k, psl = pslq.popleft()
                        et = epool.tile([128, FB], bf16, tag="e", name=f"et_{k}")
                        nc.scalar.activation(et[:], psl[:], EXP)
                        etq.append(et)

                    def pump():
                        pump_logits()
                        pump_exp()

                    pending = []

                    psW_holder = [None]

                    def outproj_group(dc, lb, on_act=False):
                        # out.T[d-chunk, l-slice] = sum_ch wo_pair.T @ ont_pair
                        ps = psW_holder[0].tile(
                            [128, 512], f32, tag="w", name=f"ps3_{dc}_{lb}"
                        )
                        for ch in range(2):
                            nc.tensor.matmul(
                                ps[:],
                                wo_sb[:, ch, dc * 128 : (dc + 1) * 128],
                                ont[:, ch, lb * 512 : (lb + 1) * 512],
                                start=(ch == 0),
                                stop=(ch == 1),
                            )
                        ot = opool.tile(
                            [128, 512], f32, tag="o", name=f"ot_{dc}_{lb}"
                        )
                        if on_act:  # drain: ACT is idle after the last exp
                            nc.scalar.copy(out=ot[:], in_=ps[:])
                        else:
                            nc.vector.tensor_copy(out=ot[:], in_=ps[:])
                        nc.sync.dma_start(
                            out=out[
                                dc * 128 : (dc + 1) * 128,
                                lb * 512 : (lb + 1) * 512,
                            ],
                            in_=ot[:],
                        )

                    # KT: half 0 (heads 0/1) plain; half 1 interleaved with
                    # the first PREF logits+exp of head 0 so the ACT spins
                    # up while KT finishes
                    grid = [(e, lb) for e in range(2) for lb in range(L // 512)]

                    globals_psK = [None]

                    def kt_half(half, interleave):
                        cells = grid[half * 4 : half * 4 + 4]
                        pss = [
                            globals_psK[0].tile(
                                [128, 512], f32, tag="psk", name=f"k_{half}_{i}"
                            )
                            for i in range(len(cells))
                        ]
                        for d in range(DCH):
                            for ps, (e, lb) in zip(pss, cells):
                                nc.tensor.matmul(
                                    ps[:],
                                    wk_sb[:, d, e * 128 : (e + 1) * 128],
                                    xtk[:, d, lb * 512 : (lb + 1) * 512],
                                    start=(d == 0),
                                    stop=(d == DCH - 1),
                                )
                            if interleave and d % 2 == 1:
                                pump_logits()
                                pump_exp()
                        for ps, (e, lb) in zip(pss, cells):
                            sl = slice(lb * 512, (lb + 1) * 512)
                            nc.vector.tensor_copy(
                                out=kt_z[0:64, 2 * e, sl], in_=ps[0:64, :]
                            )
                            nc.vector.tensor_copy(
                                out=kt_z[64:128, 2 * e + 1, sl],
                                in_=ps[64:128, :],
                            )

                    with tc.tile_pool(name="psK", bufs=4, space="PSUM") as psK:
                        globals_psK[0] = psK
                        kt_half(0, False)
                        kt_half(1, True)
                        while cursor[0] < PREF:
                            pump_logits()
                            pump_exp()

                    # V projection in 2-bank passes, interleaved into
                    # head 0's t-loop below so PV starts as soon as each
                    # pair of V t-chunks lands
                    psO_cm = tc.tile_pool(name="psO", bufs=1, space="PSUM")
                    psO = psO_cm.__enter__()
                    psV_cm = tc.tile_pool(name="psV", bufs=2, space="PSUM")
                    psV = psV_cm.__enter__()

                    def v_pass(p):
                        ts2 = range(p * 2, p * 2 + 2)
                        vps = [
                            psV.tile(
                                [128, E], f32, tag="psv", name=f"v_{p}_{i}"
                            )
                            for i in range(2)
                        ]
                        for d in range(DCH):
                            for ps, t in zip(vps, ts2):
                                nc.tensor.matmul(
                                    ps[:],
                                    xtv[:, d, t * 128 : (t + 1) * 128],
                                    wv_sb[:, d, :],
                                    start=(d == 0),
                                    stop=(d == DCH - 1),
                                )
                        for ps, t in zip(vps, ts2):
                            nc.scalar.copy(
                                out=v_sb[:, t, :].rearrange(
                                    "p (n c) -> p n c", n=HPC
                                )[:, :, 0:HDIM],
                                in_=ps[:].rearrange("p (n c) -> p n c", n=HPC),
                            )

                    for hi, (fb, n) in enumerate(heads):
                        f0 = fb * FB
                        pso = psO.tile(
                            [VW, FB], f32, tag="pso", name=f"pso_{fb}_{n}"
                        )
                        for t in range(TCH):
                            if hi == 0 and t % 2 == 0:
                                v_pass(t // 2)
                            pump_logits()
                            pump_exp()
                            et = etq.popleft()
                            for h2 in range(FB // 512):
                                nc.tensor.matmul(
                                    pso[:, h2 * 512 : (h2 + 1) * 512],
                                    v_sb[:, t, n * VW : (n + 1) * VW],
                                    et[:, h2 * 512 : (h2 + 1) * 512],
                                    start=(t == 0),
                                    stop=(t == TCH - 1),
                                )
                            if t % 4 == 2 and pending:
                                outproj_group(*pending.pop(0))

                        # normalize: O.T = O_un.T * (1/norm); norm row ->
                        # [32,32] via DMA (recip cost tracks free size),
                        # recip, back to a row, gpsimd broadcast, multiply
                        if hi == 0:
                            psV_cm.__exit__(None, None, None)
                            psW_cm = tc.tile_pool(
                                name="psW", bufs=2, space="PSUM"
                            )
                            psW_holder[0] = psW_cm.__enter__()

                        # stage pso to SBUF so the single psO slot frees
                        # immediately; normalize from the staging copy
                        stage = rpool.tile(
                            [VW, FB], f32, tag="r", name=f"stage_{fb}_{n}"
                        )
                        nc.vector.tensor_copy(out=stage[:], in_=pso[:])
                        rsq = rpool.tile([32, 2 * (FB // 32)], f32, tag="rsq")
                        nc.sync.dma_start(
                            out=rsq[:, 0 : FB // 32],
                            in_=stage[HDIM : HDIM + 1, :],
                        )
                        with nc.allow_low_precision(reason="softmax recip"):
                            nc.vector.reciprocal(
                                out=rsq[:, FB // 32 :],
                                in_=rsq[:, 0 : FB // 32],
                            )
                        rt2 = rpool.tile([1, FB], f32, tag="r2")
                        nc.sync.dma_start(out=rt2[:], in_=rsq[:, FB // 32 :])
                        bt = bpool.tile([64, FB], f32, tag="b")
                        nc.gpsimd.partition_broadcast(
                            bt[:], rt2[:], channels=64
                        )
                        ch, par = n // 2, n % 2
                        if par == 0:
                            nc.vector.tensor_mul(
                                out=ont[0:HDIM, ch, f0 : f0 + FB],
                                in0=stage[0:HDIM, :],
                                in1=bt[:],
                            )
                        else:
                            # odd head: multiply at partitions 0-63, then
                            # DMA-shift the bf16 result to partitions 64-127
                            ot2 = rpool.tile(
                                [64, FB], bf16, tag="o2", name=f"o2_{fb}_{n}"
                            )
                            nc.vector.tensor_mul(
                                out=ot2[:], in0=stage[0:HDIM, :], in1=bt[:]
                            )
                            nc.sync.dma_start(
                                out=ont[64:128, ch, f0 : f0 + FB], in_=ot2[:]
                            )

                        if n == HPC - 1:
                            pending += [
                                (dc, lb)
                                for lb in range(
                                    fb * (FB // 512), (fb + 1) * (FB // 512)
                                )
                                for dc in range(D // 128)
                            ]

                    # drain the last f-block's out-projection
                    for gi, (dc, lb) in enumerate(pending):
                        outproj_group(dc, lb, on_act=(gi % 2 == 0))
                    psW_cm.__exit__(None, None, None)
                    psO_cm.__exit__(None, None, None)

    nc.compile()
    return nc


def _get_nc():
    global _CACHED_NC
    if _CACHED_NC is None:
        _CACHED_NC = _build_nc()
    return _CACHED_NC


def _make_in_maps(query_input, key_input, value_input, Wq, Wk, Wv, Wo):
    import ml_dtypes

    bf16 = ml_dtypes.bfloat16
    scale = np.float32(HDIM) ** np.float32(-0.5)

    xT = {}
    for b in range(B):
        xT[("q", b)] = np.ascontiguousarray(query_input[b].T).astype(bf16)
        xT[("k", b)] = np.ascontiguousarray(key_input[b].T).astype(bf16)
        xT[("v", b)] = np.ascontiguousarray(value_input[b].T).astype(bf16)

    in_maps = []
    for core in range(NCORES):
        b = core // 4
        g = core % 4
        hs = slice(g * HPC, (g + 1) * HPC)
        in_maps.append(
            {
                "xq": xT[("q", b)],
                "xk": xT[("k", b)],
                "xv": xT[("v", b)],
                "wq": np.ascontiguousarray(
                    (Wq[:, hs, :] * scale).reshape(D, E)
                ).astype(bf16),
                "wk": np.ascontiguousarray(Wk[:, hs, :].reshape(D, E)).astype(bf16),
                "wv": np.ascontiguousarray(Wv[:, hs, :].reshape(D, E)).astype(bf16),
                "wo": np.ascontiguousarray(Wo[hs].reshape(E, D)).astype(bf16),
            }
        )
    return in_maps


def _combine(results):
    # device results are transposed [D, L]
    out = np.empty((B, L, D), dtype=np.float32)
    for b in range(B):
        acc = results[b * 4]["out"].astype(np.float32)
        for g in range(1, 4):
            acc = acc + results[b * 4 + g]["out"]
        out[b] = acc.T
    return out


def kernel(query_input, key_input, value_input, Wq, Wk, Wv, Wo):
    from concourse.bass_utils import run_bass_kernel_spmd

    nc = _get_nc()
    in_maps = _make_in_maps(query_input, key_input, value_input, Wq, Wk, Wv, Wo)
    res = run_bass_kernel_spmd(nc, in_maps, core_ids=list(range(NCORES)))
    return _combine(res.results)


if __name__ == "__main__":
    rng = np.random.default_rng(0)
    inputs = {
        "query_input": rng.standard_normal((B, L, D), dtype=np.float32),
        "key_input": rng.standard_normal((B, L, D), dtype=np.float32),
        "value_input": rng.standard_normal((B, L, D), dtype=np.float32),
        "Wq": rng.standard_normal((D, NHEADS, HDIM), dtype=np.float32) * 0.03,
        "Wk": rng.standard_normal((D, NHEADS, HDIM), dtype=np.float32) * 0.03,
        "Wv": rng.standard_normal((D, NHEADS, HDIM), dtype=np.float32) * 0.03,
        "Wo": rng.standard_normal((NHEADS, HDIM, D), dtype=np.float32) * 0.03,
    }
    out = kernel(**inputs)
    print("kernel output", out.shape, out.dtype, float(np.abs(out).mean()))



# revision 22
# speedup vs baseline: 1.2695x; 1.0352x over previous
"""Multi-head attention on 8 Trainium2 NeuronCores.

Problem: B=2, L=2048, D=1024, N=16 heads, H=64.
Sharding: core i -> batch (i // 4), heads [4*(i%4), 4*(i%4)+4).
Each core: QKV projections for its 4 heads, full-seq attention,
partial output projection. Host sums the 4 partial projections per batch.

Device pipeline (per core), all layouts chosen so the contraction dim is
on partitions (no on-device transposes; host passes x pre-transposed):
  QT[e,l] = sum_d wq[d,e] xT[d,l]      (lhsT=wq chunk, rhs=xT chunk)
  KT[e,l] likewise; V[t,e] = sum_d xT[d,t-chunk] wv[d,e] (lhsT=xT, rhs=wv)
  LT[t,f] = sum_h KT[h,t] QT[h,f]      (per head)
  E = exp(LT)                          (ACT, PSUM->SBUF)
  [O_un.T ; norm] = [V_n | 1].T @ E:   lhsT=[V_n|ones][t,65], rhs=E[t,f]
  O.T = O_un.T * (1/norm) broadcast    (rank-1 matmul + 64-lane recip)
  out[l,d] += sum_h O.T[h,l-chunk] wo[h,d]

K=64 matmuls run at half rate on TRN2, so all contraction-64 matmuls
(logits, out-proj) are padded to K=128: the stationary operand keeps its
head's 64 rows and zeros elsewhere, making the moving operand's other
rows irrelevant. Matmuls run as float32r (full PE rate, ~TF32 rounding);
x and all weights are bf16 host-cast.
"""

import numpy as np

B, L, D = 2, 2048, 1024
NHEADS, HDIM = 16, 64
NCORES = 8
HPC = 4  # heads per core
E = HPC * HDIM  # 256
DCH = D // 128  # 8 d-chunks
TCH = L // 128  # 16 t/l chunks
FB = 1024  # f-block size in attention phase
NFB = L // FB
VW = HDIM + 1  # V' width per head (64 cols V + 1 ones col)

_CACHED_NC = None


def _build_nc():
    import concourse.mybir as mybir
    from concourse import bacc
    from concourse.tile import TileContext

    f32 = mybir.dt.float32
    f32r = mybir.dt.float32r
    bf16 = mybir.dt.bfloat16
    EXP = mybir.ActivationFunctionType.Exp

    nc = bacc.Bacc("TRN2", target_bir_lowering=False, num_devices=NCORES)

    xq = nc.declare_dram_parameter("xq", [D, L], bf16, isOutput=False)
    xk = nc.declare_dram_parameter("xk", [D, L], bf16, isOutput=False)
    xv = nc.declare_dram_parameter("xv", [D, L], bf16, isOutput=False)
    wq = nc.declare_dram_parameter("wq", [D, E], bf16, isOutput=False)
    wk = nc.declare_dram_parameter("wk", [D, E], bf16, isOutput=False)
    wv = nc.declare_dram_parameter("wv", [D, E], bf16, isOutput=False)
    wo = nc.declare_dram_parameter("wo", [E, D], bf16, isOutput=False)
    # out is stored TRANSPOSED [D, L]; host transposes back. This lets the
    # out-projection run as out.T[d,l] = sum_e wo[e,d] ont[e,l] with TWO
    # heads stacked in the K dim (no zero padding): 2 matmuls per tile
    # instead of 4.
    out = nc.declare_dram_parameter("out", [D, L], bf16, isOutput=True)

    with TileContext(nc) as tc:
        with tc.tile_pool(name="persist", bufs=1) as cpool:
            # --- persistent SBUF tensors ---
            wq_sb = cpool.tile([128, DCH, E], bf16, tag="wq")
            wk_sb = cpool.tile([128, DCH, E], bf16, tag="wk")
            wv_sb = cpool.tile([128, DCH, E], bf16, tag="wv")
            # wo pair-stacked: pair ch holds head 2ch in rows 0-63 and head
            # 2ch+1 in rows 64-127 -> K=128 out-proj with no padding
            wo_sb = cpool.tile([128, 2, D], bf16, tag="wo")
            qt_sb = cpool.tile([128, 2, L], f32r, tag="qt")
            # kt zero-padded per head: head n's data in rows (n%2)*64..+64,
            # zeros in the other 64 rows -> K=128 logits at full rate
            kt_z = cpool.tile([128, HPC, L], f32r, tag="ktz")
            v_sb = cpool.tile([128, TCH, HPC * VW], bf16, tag="v")
            # O.T pair-stacked like wo_sb
            ont = cpool.tile([128, 2, L], bf16, tag="ont")

            # ---------------- Phase 1: KT + partial QT ----------------
            # DMA order is latency-tuned: wk, xk (lb-major) gate KT; then
            # xq gates the first QT cells; xv streams in behind the pump.
            # lb-major transfers let each KT cell start as soon as its
            # l-slice lands instead of waiting for the whole tensor.
            with tc.tile_pool(name="xp", bufs=2) as xpool:

                def load_x(x_dram):
                    xt = xpool.tile([128, DCH, L], bf16, tag="x")
                    xr = x_dram.rearrange("(c p) l -> p c l", p=128)
                    for d in range(DCH):
                        nc.sync.dma_start(out=xt[:, d, :], in_=xr[:, d, :])
                    return xt

                with tc.tile_pool(name="psA", bufs=4, space="PSUM") as psA:
                    # tiny constant FIRST in the DMA queue: the kt_z zero
                    # stripes (DVE) depend on it and gate the first logits
                    cst = np.concatenate(
                        [
                            np.ones((128, 64), np.float32),
                            np.zeros((128, 1), np.float32),
                        ],
                        axis=1,
                    )
                    cst_dram = nc.inline_tensor(cst, name="cst")
                    cst_src = cst_dram.ap().bitcast(f32r)
                    zero_sb = cpool.tile([128, 1], f32r, tag="zero")
                    nc.sync.dma_start(out=zero_sb[:], in_=cst_src[:, 64:65])
                    for zn in range(HPC):
                        lo = 64 if zn % 2 == 0 else 0
                        nc.vector.tensor_copy(
                            out=kt_z[lo : lo + 64, zn, :],
                            in_=zero_sb[lo : lo + 64, 0:1].to_broadcast(
                                (64, L)
                            ),
                        )
                    nc.vector.memset(
                        v_sb[:].rearrange("p t (n c) -> p t n c", n=HPC)[
                            :, :, :, HDIM : HDIM + 1
                        ],
                        1.0,
                    )

                    nc.sync.dma_start(
                        out=wk_sb[:],
                        in_=wk.rearrange("(c p) e -> p c e", p=128),
                    )
                    xtk = load_x(xk)
                    nc.sync.dma_start(
                        out=wq_sb[:],
                        in_=wq.rearrange("(c p) e -> p c e", p=128),
                    )
                    xtq = xpool.tile([128, DCH, L], bf16, tag="x")
                    xqr = xq.rearrange("(c p) l -> p c l", p=128)
                    for half in range(2):
                        sl = slice(half * 1024, (half + 1) * 1024)
                        for d in range(DCH):
                            nc.sync.dma_start(
                                out=xtq[:, d, sl], in_=xqr[:, d, sl]
                            )

                    def qt_cells(pool, cells, tagi, pump_fn=None):
                        pss = [
                            pool.tile(
                                [128, 512], f32, tag="ps",
                                name=f"q_{tagi}_{i}",
                            )
                            for i in range(len(cells))
                        ]
                        for d in range(DCH):
                            for ps, (e, lb) in zip(pss, cells):
                                nc.tensor.matmul(
                                    ps[:],
                                    wq_sb[:, d, e * 128 : (e + 1) * 128],
                                    xtq[:, d, lb * 512 : (lb + 1) * 512],
                                    start=(d == 0),
                                    stop=(d == DCH - 1),
                                )
                            if pump_fn is not None and d % 2 == 1:
                                pump_fn()
                        for ps, (e, lb) in zip(pss, cells):
                            nc.vector.tensor_copy(
                                out=qt_sb[:, e, lb * 512 : (lb + 1) * 512],
                                in_=ps[:],
                            )

                    def kt_cells(pool, e, tagi, pump_fn=None):
                        cells = list(range(4))
                        pss = [
                            pool.tile(
                                [128, 512], f32, tag="ps", name=f"k_{tagi}_{i}"
                            )
                            for i in cells
                        ]
                        for d in range(DCH):
                            for ps, lb in zip(pss, cells):
                                nc.tensor.matmul(
                                    ps[:],
                                    wk_sb[:, d, e * 128 : (e + 1) * 128],
                                    xtk[:, d, lb * 512 : (lb + 1) * 512],
                                    start=(d == 0),
                                    stop=(d == DCH - 1),
                                )
                            if pump_fn is not None and d % 2 == 1:
                                pump_fn()
                        for ps, lb in zip(pss, cells):
                            sl = slice(lb * 512, (lb + 1) * 512)
                            nc.vector.tensor_copy(
                                out=kt_z[0:64, 2 * e, sl], in_=ps[0:64, :]
                            )
                            nc.vector.tensor_copy(
                                out=kt_z[64:128, 2 * e + 1, sl],
                                in_=ps[64:128, :],
                            )

                    # KT e-chunk 0 (pair 0): gates pump start; e=1 runs in
                    # the pump warmup
                    kt_cells(psA, 0, 0)

                    # QT for f-block 0 (lbs 0-1): gates pump start; the
                    # rest of QT runs interleaved into the pump warmup
                    qt_cells(
                        psA, [(e, lb) for e in range(2) for lb in (0, 1)], 0
                    )

                    xtv = load_x(xv)
                    nc.sync.dma_start(
                        out=wv_sb[:],
                        in_=wv.rearrange("(c p) e -> p c e", p=128),
                    )
                    # pair-stacked wo: row index = (2*ch + par)*64 + h
                    wor = wo.rearrange(
                        "(ch par h) d -> h ch par d", ch=2, par=2
                    )
                    nc.sync.dma_start(
                        out=wo_sb[0:64, :, :], in_=wor[:, :, 0, :]
                    )
                    nc.sync.dma_start(
                        out=wo_sb[64:128, :, :], in_=wor[:, :, 1, :]
                    )

                # ---- Phase 2(+KT,+out-proj): ACT-paced global pump ----
                # The logits->exp stream runs through a global cursor with
                # ~PREF tiles of lookahead, so the ACT engine never stalls
                # on PE hiccups, head boundaries, or the KT projection
                # (which runs here, interleaved, on its own PSUM pool).
                PREF = 14
                from collections import deque

                with (
                    tc.tile_pool(name="psL", bufs=2, space="PSUM") as psL,
                    tc.tile_pool(name="ep", bufs=PREF) as epool,
                    tc.tile_pool(name="rp", bufs=2) as rpool,
                    tc.tile_pool(name="bp", bufs=2) as bpool,
                    tc.tile_pool(name="op", bufs=3) as opool,
                ):
                    heads = [(fb, n) for fb in range(NFB) for n in range(HPC)]
                    cursor = [0]
                    pslq = deque()
                    etq = deque()

                    def pump_logits():
                        k = cursor[0]
                        if k >= len(heads) * TCH:
                            return
                        cursor[0] += 1
                        fb, n = heads[k // TCH]
                        t = k % TCH
                        psl = psL.tile(
                            [128, FB], f32, tag="psl", name=f"psl_{k}"
                        )
                        ch, f0 = n // 2, fb * FB
                        for h2 in range(FB // 512):
                            nc.tensor.matmul(
                                psl[:, h2 * 512 : (h2 + 1) * 512],
                                kt_z[:, n, t * 128 : (t + 1) * 128],
                                qt_sb[
                                    :, ch, f0 + h2 * 512 : f0 + (h2 + 1) * 512
                                ],
                                start=True,
                                stop=True,
                            )
                        pslq.append((k, psl))

                    def pump_exp():
                        if not pslq:
                            return
                        k, psl = pslq.popleft()
                        et = epool.tile([128, FB], bf16, tag="e", name=f"et_{k}")
                        nc.scalar.activation(et[:], psl[:], EXP)
                        etq.append(et)

                    def pump():
                        pump_logits()
                        pump_exp()

                    pending = []

                    psW_holder = [None]

                    def outproj_group(dc, lb, on_act=False):
                        # out.T[d-chunk, l-slice] = sum_ch wo_pair.T @ ont_pair
                        ps = psW_holder[0].tile(
                            [128, 512], f32, tag="w", name=f"ps3_{dc}_{lb}"
                        )
                        for ch in range(2):
                            nc.tensor.matmul(
                                ps[:],
                                wo_sb[:, ch, dc * 128 : (dc + 1) * 128],
                                ont[:, ch, lb * 512 : (lb + 1) * 512],
                                start=(ch == 0),
                                stop=(ch == 1),
                            )
                        ot = opool.tile(
                            [128, 512], bf16, tag="o", name=f"ot_{dc}_{lb}"
                        )
                        if on_act:  # drain: ACT is idle after the last exp
                            nc.scalar.copy(out=ot[:], in_=ps[:])
                        else:
                            nc.vector.tensor_copy(out=ot[:], in_=ps[:])
                        nc.sync.dma_start(
                            out=out[
                                dc * 128 : (dc + 1) * 128,
                                lb * 512 : (lb + 1) * 512,
                            ],
                            in_=ot[:],
                        )

                    # Warmup: prime PREF pump steps while the remaining
                    # QT cells (f-blocks 2-3) run on a scratch PSUM pool.
                    with tc.tile_pool(name="psK", bufs=4, space="PSUM") as psK:
                        kt_cells(psK, 1, 1, pump_fn=pump)
                        qt_cells(
                            psK,
                            [(e, lb) for e in range(2) for lb in (2, 3)],
                            1,
                            pump_fn=pump,
                        )
                        while cursor[0] < PREF:
                            pump()

                    # V projection in 2-bank passes, interleaved into
                    # head 0's t-loop below so PV starts as soon as each
                    # pair of V t-chunks lands
                    psO_cm = tc.tile_pool(name="psO", bufs=1, space="PSUM")
                    psO = psO_cm.__enter__()
                    psV_cm = tc.tile_pool(name="psV", bufs=2, space="PSUM")
                    psV = psV_cm.__enter__()

                    def v_pass(p):
                        ts2 = range(p * 2, p * 2 + 2)
                        vps = [
                            psV.tile(
                                [128, E], f32, tag="psv", name=f"v_{p}_{i}"
                            )
                            for i in range(2)
                        ]
                        for d in range(DCH):
                            for ps, t in zip(vps, ts2):
                                nc.tensor.matmul(
                                    ps[:],
                                    xtv[:, d, t * 128 : (t + 1) * 128],
                                    wv_sb[:, d, :],
                                    start=(d == 0),
                                    stop=(d == DCH - 1),
                                )
                        for ps, t in zip(vps, ts2):
                            nc.scalar.copy(
                                out=v_sb[:, t, :].rearrange(
                                    "p (n c) -> p n c", n=HPC
                                )[:, :, 0:HDIM],
                                in_=ps[:].rearrange("p (n c) -> p n c", n=HPC),
                            )

                    for hi, (fb, n) in enumerate(heads):
                        f0 = fb * FB
                        pso = psO.tile(
                            [VW, FB], f32, tag="pso", name=f"pso_{fb}_{n}"
                        )
                        for t in range(TCH):
                            if hi == 0 and t % 2 == 0:
                                v_pass(t // 2)
                            pump_logits()
                            pump_exp()
                            et = etq.popleft()
                            for h2 in range(FB // 512):
                                nc.tensor.matmul(
                                    pso[:, h2 * 512 : (h2 + 1) * 512],
                                    v_sb[:, t, n * VW : (n + 1) * VW],
                                    et[:, h2 * 512 : (h2 + 1) * 512],
                                    start=(t == 0),
                                    stop=(t == TCH - 1),
                                )
                            if t % 4 == 2 and pending:
                                outproj_group(*pending.pop(0))

                        # normalize: O.T = O_un.T * (1/norm); norm row ->
                        # [32,32] via DMA (recip cost tracks free size),
                        # recip, back to a row, gpsimd broadcast, multiply
                        if hi == 0:
                            psV_cm.__exit__(None, None, None)
                            psW_cm = tc.tile_pool(
                                name="psW", bufs=2, space="PSUM"
                            )
                            psW_holder[0] = psW_cm.__enter__()

                        # stage pso to SBUF so the single psO slot frees
                        # immediately; normalize from the staging copy
                        stage = rpool.tile(
                            [VW, FB], f32, tag="r", name=f"stage_{fb}_{n}"
                        )
                        nc.vector.tensor_copy(out=stage[:], in_=pso[:])
                        rsq = rpool.tile([32, 2 * (FB // 32)], f32, tag="rsq")
                        nc.sync.dma_start(
                            out=rsq[:, 0 : FB // 32],
                            in_=stage[HDIM : HDIM + 1, :],
                        )
                        with nc.allow_low_precision(reason="softmax recip"):
                            nc.vector.reciprocal(
                                out=rsq[:, FB // 32 :],
                                in_=rsq[:, 0 : FB // 32],
                            )
                        rt2 = rpool.tile([1, FB], f32, tag="r2")
                        nc.sync.dma_start(out=rt2[:], in_=rsq[:, FB // 32 :])
                        bt = bpool.tile([64, FB], f32, tag="b")
                        nc.gpsimd.partition_broadcast(
                            bt[:], rt2[:], channels=64
                        )
                        ch, par = n // 2, n % 2
                        if par == 0:
                            nc.vector.tensor_mul(
                                out=ont[0:HDIM, ch, f0 : f0 + FB],
                                in0=stage[0:HDIM, :],
                                in1=bt[:],
                            )
                        else:
                            # odd head: multiply at partitions 0-63, then
                            # DMA-shift the bf16 result to partitions 64-127
                            ot2 = rpool.tile(
                                [64, FB], bf16, tag="o2", name=f"o2_{fb}_{n}"
                            )
                            nc.vector.tensor_mul(
                                out=ot2[:], in0=stage[0:HDIM, :], in1=bt[:]
                            )
                            nc.sync.dma_start(
                                out=ont[64:128, ch, f0 : f0 + FB], in_=ot2[:]
                            )

                        if n == HPC - 1:
                            pending += [
                                (dc, lb)
                                for lb in range(
                                    fb * (FB // 512), (fb + 1) * (FB // 512)
                                )
                                for dc in range(D // 128)
                            ]

                    # drain the last f-block's out-projection
                    for gi, (dc, lb) in enumerate(pending):
                        outproj_group(dc, lb, on_act=(gi % 2 == 0))
                    psW_cm.__exit__(None, None, None)
                    psO_cm.__exit__(None, None, None)

    nc.compile()
    return nc


def _get_nc():
    global _CACHED_NC
    if _CACHED_NC is None:
        _CACHED_NC = _build_nc()
    return _CACHED_NC


def _make_in_maps(query_input, key_input, value_input, Wq, Wk, Wv, Wo):
    import ml_dtypes

    bf16 = ml_dtypes.bfloat16
    scale = np.float32(HDIM) ** np.float32(-0.5)

    xT = {}
    for b in range(B):
        xT[("q", b)] = np.ascontiguousarray(query_input[b].T).astype(bf16)
        xT[("k", b)] = np.ascontiguousarray(key_input[b].T).astype(bf16)
        xT[("v", b)] = np.ascontiguousarray(value_input[b].T).astype(bf16)

    in_maps = []
    for core in range(NCORES):
        b = core // 4
        g = core % 4
        hs = slice(g * HPC, (g + 1) * HPC)
        in_maps.append(
            {
                "xq": xT[("q", b)],
                "xk": xT[("k", b)],
                "xv": xT[("v", b)],
                "wq": np.ascontiguousarray(
                    (Wq[:, hs, :] * scale).reshape(D, E)
                ).astype(bf16),
                "wk": np.ascontiguousarray(Wk[:, hs, :].reshape(D, E)).astype(bf16),
                "wv": np.ascontiguousarray(Wv[:, hs, :].reshape(D, E)).astype(bf16),
                "wo": np.ascontiguousarray(Wo[hs].reshape(E, D)).astype(bf16),
            }
        )
    return in_maps


def _combine(results):
    # device results are transposed [D, L]
    out = np.empty((B, L, D), dtype=np.float32)
    for b in range(B):
        acc = results[b * 4]["out"].astype(np.float32)
        for g in range(1, 4):
            acc = acc + results[b * 4 + g]["out"]
        out[b] = acc.T
    return out


def kernel(query_input, key_input, value_input, Wq, Wk, Wv, Wo):
    from concourse.bass_utils import run_bass_kernel_spmd

    nc = _get_nc()
    in_maps = _make_in_maps(query_input, key_input, value_input, Wq, Wk, Wv, Wo)
    res = run_bass_kernel_spmd(nc, in_maps, core_ids=list(range(NCORES)))
    return _combine(res.results)


if __name__ == "__main__":
    rng = np.random.default_rng(0)
    inputs = {
        "query_input": rng.standard_normal((B, L, D), dtype=np.float32),
        "key_input": rng.standard_normal((B, L, D), dtype=np.float32),
        "value_input": rng.standard_normal((B, L, D), dtype=np.float32),
        "Wq": rng.standard_normal((D, NHEADS, HDIM), dtype=np.float32) * 0.03,
        "Wk": rng.standard_normal((D, NHEADS, HDIM), dtype=np.float32) * 0.03,
        "Wv": rng.standard_normal((D, NHEADS, HDIM), dtype=np.float32) * 0.03,
        "Wo": rng.standard_normal((NHEADS, HDIM, D), dtype=np.float32) * 0.03,
    }
    out = kernel(**inputs)
    print("kernel output", out.shape, out.dtype, float(np.abs(out).mean()))

